# revision 26
# baseline (speedup 1.0000x reference)
"""Trainium2 Bass kernel for nn_Attention_23433341567267 (sparse_attention).

5 masked-softmax score pipelines over (B=8, H=12, S=512, D=64) plus one
attention-output matmul.  Sharded: core b handles batch b (all 12 heads).

All pipelines are computed k-major (transposed scores):
  sT[k,q] = B[k,:].A[q,:] + maskbiasT[k,q]  (PE: score MM head-pair row-packed
                                             + bf16 identity@maskT inject,
                                             both accumulated in PSUM)
  ET      = exp(sT)                          (ACT -> bf16, 2-chunk ops)
  sums[q]: pipeline 0 via V'=[V|1] ones column inside the PV matmul
           (outT'[65,512] row 64); pipelines 1-4 via ones-column matmuls
           col-packed 4-wide into one PSUM bank (rows 0/32/64/96)
  rec     = 1/sums            (DVE row reciprocal -> bf16)
  pbs     = broadcast(rec)    (GPSIMD partition_broadcast, SBUF bf16)
  PT      = ET * pbs          (DVE/GPSIMD tensor_mul, bf16; DMA out)
  outT    = outT'[0:64] * pbs (ACT copy + DVE mul, f32; DMA out)

Host side does sharding/layout only: d-major transposes, 1/sqrt(D) folded
into the q-side operands, bf16 storage conversion, mask -> additive bias,
and transpose-back + f32 upcast of gathered outputs.
"""

import numpy as np
import ml_dtypes

B, H, S, D = 8, 12, 512, 64
NCORES = 8
KC = S // 128  # 128-chunks per sequence
# (A_idx, B_idx) into the stacked operand tensor
# [0]=qT*scale [1]=kT [2]=xo1T*scale [3]=xo2T [4]=xp1T*scale [5]=xp2T
PIPES = [(0, 1), (2, 3), (2, 5), (4, 3), (4, 5)]
BF16_OPS = True   # score operands in bf16 (f32r otherwise)
POOL_MULS = 0     # of every 10 normalize muls, this many go to GPSIMD

_CACHE = {}


def _build_nc():
    import concourse.mybir as mybir
    import concourse.tile as tile
    from concourse import bacc
    from concourse.bass import ts

    f32 = mybir.dt.float32
    f32r = mybir.dt.float32r
    bf16 = mybir.dt.bfloat16
    opdt = bf16 if BF16_OPS else f32r
    Exp = mybir.ActivationFunctionType.Exp

    nc = bacc.Bacc("TRN2", target_bir_lowering=False, debug=False,
                   num_devices=NCORES)
    opsT = nc.declare_dram_parameter("opsT", [6, H, D, S], opdt, isOutput=False)
    vext = nc.declare_dram_parameter("vext", [H, S, D + 1], bf16, isOutput=False)
    maskTb = nc.declare_dram_parameter("maskTb", [S, S], bf16, isOutput=False)
    consts = nc.declare_dram_parameter("consts", [128, 256], bf16, isOutput=False)
    pT = nc.declare_dram_parameter("pT", [5, H, 128, KC, S], bf16, isOutput=True)
    outT = nc.declare_dram_parameter("outT", [H, D, S], f32, isOutput=True)

    n_mul = 0  # running index to split normalize muls DVE/Pool

    with tile.TileContext(nc) as tc:
        with (
            tc.tile_pool(name="const", bufs=1) as const_pool,
            tc.tile_pool(name="ops", bufs=3) as ops_pool,
            tc.tile_pool(name="v", bufs=3) as v_pool,
            tc.tile_pool(name="et", bufs=15) as et_pool,
            tc.tile_pool(name="pt", bufs=12) as pt_pool,
            tc.tile_pool(name="rec", bufs=8) as rec_pool,
            tc.tile_pool(name="pbs", bufs=8) as pbs_pool,
            tc.tile_pool(name="oc", bufs=2) as oc_pool,
            tc.tile_pool(name="on", bufs=2) as on_pool,
            tc.tile_pool(name="ps", bufs=3, space="PSUM") as ps_pool,
            tc.tile_pool(name="po", bufs=1, space="PSUM") as po_pool,
            tc.tile_pool(name="su", bufs=1, space="PSUM") as su_pool,
        ):
            ct = const_pool.tile([128, 256], bf16)
            nc.sync.dma_start(ct[:], consts[:])
            mt = const_pool.tile([128, KC, S], bf16)
            mview = maskTb.rearrange("(c p) q -> p c q", p=128)
            nc.sync.dma_start(mt[:, 0:2, :], mview[:, 0:2, :])
            nc.sync.dma_start(mt[:, 2:4, :], mview[:, 2:4, :])
            ident = ct[:, 0:128]
            ones_col = ct[:, 128:129]

            for hp in range(H // 2):
                ops6 = ops_pool.tile([128, 6, S], opdt, tag="ops")
                nc.sync.dma_start(
                    ops6[:, 0:3, :],
                    opsT[0:3, 2 * hp : 2 * hp + 2].rearrange("t a p f -> (a p) t f"),
                )
                nc.sync.dma_start(
                    ops6[:, 3:6, :],
                    opsT[3:6, 2 * hp : 2 * hp + 2].rearrange("t a p f -> (a p) t f"),
                )
                v8 = v_pool.tile([128, 2, KC, D + 1], bf16, tag="v")
                nc.sync.dma_start(
                    v8[:],
                    vext[2 * hp : 2 * hp + 2].rearrange("h (c p) d -> p h c d", p=128),
                )
                for hh in range(2):
                    h = 2 * hp + hh
                    lo, hi = 64 * hh, 64 * hh + 64
                    po = po_pool.tile([D + 1, S], f32, tag="po")
                    su = su_pool.tile([128, S], f32, tag="su")
                    for p, (ia, ib) in enumerate(PIPES):
                        et4 = et_pool.tile([128, KC, S], bf16, tag="et",
                                           name=f"et4_{p}")
                        for half in range(2):
                            ps = ps_pool.tile([128, 2, S], f32, tag="ps")
                            for sub in range(2):
                                kc = 2 * half + sub
                                nc.tensor.matmul(
                                    ps[:, sub, :], lhsT=ident, rhs=mt[:, kc, :],
                                    start=True, stop=False,
                                )
                                nc.tensor.matmul(
                                    ps[:, sub, :],
                                    lhsT=ops6[lo:hi, ib, ts(kc, 128)],
                                    rhs=ops6[lo:hi, ia, :],
                                    start=False, stop=True,
                                )
                            nc.scalar.activation(
                                et4[:, 2 * half : 2 * half + 2, :], ps[:], Exp
                            )
                        if p == 0:
                            for kc in range(KC):
                                nc.tensor.matmul(
                                    po[:], lhsT=v8[:, hh, kc, :],
                                    rhs=et4[:, kc, :],
                                    start=(kc == 0), stop=(kc == KC - 1),
                                )
                            sums_row = po[D : D + 1, :]
                        else:
                            j = p - 1
                            for kc in range(KC):
                                nc.tensor.matmul(
                                    su[32 * j : 32 * j + 1, :], lhsT=ones_col,
                                    rhs=et4[:, kc, :],
                                    start=(kc == 0), stop=(kc == KC - 1),
                                    tile_position=(0, 32 * j),
                                )
                            sums_row = su[32 * j : 32 * j + 1, :]
                        rec_f = rec_pool.tile([1, S], f32, tag="recf",
                                              name=f"rec_f{p}")
                        if p == 1:
                            nc.vector.reciprocal_approx_fast(rec_f[:], sums_row)
                        else:
                            sums_sb = rec_pool.tile([1, S], f32, tag="sums",
                                                    name=f"sums_sb{p}")
                            nc.vector.tensor_copy(sums_sb[:], sums_row)
                            nc.vector.reciprocal_approx_fast(rec_f[:], sums_sb[:])
                        rec = rec_pool.tile([1, S], bf16, tag="rec",
                                            name=f"rec_{p}")
                        nc.gpsimd.tensor_copy(rec[:], rec_f[:])
                        pbs = pbs_pool.tile([128, S], bf16, tag="pbs",
                                            name=f"pbs_{p}")
                        nc.gpsimd.partition_broadcast(pbs[:], rec[:])
                        pt4 = pt_pool.tile([128, KC, S], bf16, tag="pt",
                                           name=f"pt4_{p}")
                        nc.vector.tensor_mul(
                            pt4[:], et4[:],
                            pbs[:, None, :].broadcast_to([128, KC, S]),
                        )
                        nc.sync.dma_start(pT[p, h][:, 0:2, :], pt4[:, 0:2, :])
                        nc.sync.dma_start(pT[p, h][:, 2:4, :], pt4[:, 2:4, :])
                        if p == 0:
                            oc = oc_pool.tile([D, S], f32, tag="oc")
                            nc.scalar.copy(oc[:], po[0:D, :])
                            on = on_pool.tile([D, S], f32, tag="on")
                            nc.vector.tensor_mul(on[:], oc[:], pbs[0:D, :])
                            nc.sync.dma_start(outT[h], on[:])
    nc.finalize()
    return nc


def _get_nc():
    if "nc" not in _CACHE:
        _CACHE["nc"] = _build_nc()
    return _CACHE["nc"]


def _prep_core_inputs(b, query, key, value, mask, x_original1, x_original2,
                      x_position1, x_position2):
    bf = ml_dtypes.bfloat16
    opdt = bf if BF16_OPS else np.float32
    scale = np.float32(1.0 / np.sqrt(D))

    def t_(x, s=False):
        x = x[b].astype(np.float32)
        if s:
            x = x * scale
        return np.ascontiguousarray(x.transpose(0, 2, 1))  # [H, D, S]

    opsT = np.ascontiguousarray(np.stack([
        t_(query, True), t_(key),
        t_(x_original1, True), t_(x_original2),
        t_(x_position1, True), t_(x_position2),
    ]).astype(opdt))  # [6, H, D, S]
    vext = np.ascontiguousarray(np.concatenate(
        [value[b].astype(np.float32), np.ones((H, S, 1), np.float32)], axis=-1
    ).astype(bf))  # [H, S, D+1]
    mb = (mask[b, 0].astype(np.float32) - 1.0) * np.float32(1e9)  # [q, k]
    maskTb = np.ascontiguousarray(mb.T.astype(bf))
    consts = np.ascontiguousarray(np.concatenate(
        [np.eye(128, dtype=np.float32), np.ones((128, 128), np.float32)], axis=1
    ).astype(bf))
    return dict(opsT=opsT, vext=vext, maskTb=maskTb, consts=consts)


def kernel(query, key, value, mask, x_original1, x_original2, x_position1,
           x_position2, _run_kwargs=None):
    from concourse.bass_utils import run_bass_kernel_spmd

    nc = _get_nc()
    in_maps = [
        _prep_core_inputs(b, query, key, value, mask, x_original1, x_original2,
                          x_position1, x_position2)
        for b in range(B)
    ]
    kw = _run_kwargs or {}
    res = run_bass_kernel_spmd(nc, in_maps, list(range(NCORES)), **kw)
    _CACHE["last_result"] = res

    out = np.empty((B, H, S, D), np.float32)
    ps = [np.empty((B, H, S, S), np.float32) for _ in range(5)]
    for b in range(B):
        r = res.results[b]
        out[b] = np.asarray(r["outT"]).transpose(0, 2, 1)
        pTb = np.asarray(r["pT"]).astype(np.float32)  # [5, H, 128, KC, S]
        # k-major: pTb[j,h,p,c,q] = P[q, c*128+p]
        for j in range(5):
            ps[j][b] = pTb[j].transpose(0, 3, 2, 1).reshape(H, S, S)
    return (out, ps[0], ps[1], ps[2], ps[3], ps[4])


# revision 29
# speedup vs baseline: 1.0429x; 1.0429x over previous
"""Trainium2 Bass kernel for nn_Attention_23433341567267 (sparse_attention).

5 masked-softmax score pipelines over (B=8, H=12, S=512, D=64) plus one
attention-output matmul.  Sharded: core b handles batch b (all 12 heads).

All pipelines are computed k-major (transposed scores):
  sT[k,q] = B[k,:].A[q,:] + maskbiasT[k,q]  (PE: score MM head-pair row-packed
                                             + bf16 identity@maskT inject,
                                             both accumulated in PSUM)
  ET      = exp(sT)                          (ACT -> bf16, 2-chunk ops)
  sums[q]: pipeline 0 via V'=[V|1] ones column inside the PV matmul
           (outT'[65,512] row 64); pipelines 1-4 via ones-column matmuls
           col-packed 4-wide into one PSUM bank (rows 0/32/64/96)
  rec     = 1/sums            (DVE row reciprocal -> bf16)
  pbs     = broadcast(rec)    (GPSIMD partition_broadcast, SBUF bf16)
  PT      = ET * pbs          (DVE/GPSIMD tensor_mul, bf16; DMA out)
  outT    = outT'[0:64] * pbs (ACT copy + DVE mul, f32; DMA out)

Host side does sharding/layout only: d-major transposes, 1/sqrt(D) folded
into the q-side operands, bf16 storage conversion, mask -> additive bias,
and transpose-back + f32 upcast of gathered outputs.
"""

import numpy as np
import ml_dtypes

B, H, S, D = 8, 12, 512, 64
NCORES = 8
KC = S // 128  # 128-chunks per sequence
# (A_idx, B_idx) into the stacked operand tensor
# [0]=qT*scale [1]=kT [2]=xo1T*scale [3]=xo2T [4]=xp1T*scale [5]=xp2T
PIPES = [(0, 1), (2, 3), (2, 5), (4, 3), (4, 5)]
BF16_OPS = True   # score operands in bf16 (f32r otherwise)
POOL_MULS = 0     # of every 10 normalize muls, this many go to GPSIMD

_CACHE = {}


def _build_nc():
    import concourse.mybir as mybir
    import concourse.tile as tile
    from concourse import bacc
    from concourse.bass import ts

    f32 = mybir.dt.float32
    f32r = mybir.dt.float32r
    bf16 = mybir.dt.bfloat16
    opdt = bf16 if BF16_OPS else f32r
    Exp = mybir.ActivationFunctionType.Exp

    nc = bacc.Bacc("TRN2", target_bir_lowering=False, debug=False,
                   num_devices=NCORES)
    opsT = nc.declare_dram_parameter("opsT", [6, H, D, S], opdt, isOutput=False)
    vext = nc.declare_dram_parameter("vext", [H, S, D + 1], bf16, isOutput=False)
    maskTb = nc.declare_dram_parameter("maskTb", [S, S], bf16, isOutput=False)
    consts = nc.declare_dram_parameter("consts", [128, 256], bf16, isOutput=False)
    pT = nc.declare_dram_parameter("pT", [5, H, 128, KC, S], bf16, isOutput=True)
    outT = nc.declare_dram_parameter("outT", [H, D, S], f32, isOutput=True)

    n_mul = 0  # running index to split normalize muls DVE/Pool

    with tile.TileContext(nc) as tc:
        with (
            tc.tile_pool(name="const", bufs=1) as const_pool,
            tc.tile_pool(name="ops", bufs=3) as ops_pool,
            tc.tile_pool(name="v", bufs=3) as v_pool,
            tc.tile_pool(name="et", bufs=15) as et_pool,
            tc.tile_pool(name="pt", bufs=12) as pt_pool,
            tc.tile_pool(name="rec", bufs=8) as rec_pool,
            tc.tile_pool(name="pbs", bufs=8) as pbs_pool,
            tc.tile_pool(name="oc", bufs=2) as oc_pool,
            tc.tile_pool(name="on", bufs=2) as on_pool,
            tc.tile_pool(name="ps", bufs=3, space="PSUM") as ps_pool,
            tc.tile_pool(name="po", bufs=1, space="PSUM") as po_pool,
            tc.tile_pool(name="su", bufs=1, space="PSUM") as su_pool,
        ):
            ct = const_pool.tile([128, 256], bf16)
            nc.sync.dma_start(ct[:], consts[:])
            mt = const_pool.tile([128, KC, S], bf16)
            nc.sync.dma_start(mt[:], maskTb.rearrange("(c p) q -> p c q", p=128))
            ident = ct[:, 0:128]
            ones_col = ct[:, 128:129]

            for hp in range(H // 2):
                ops6 = ops_pool.tile([128, 6, S], opdt, tag="ops")
                nc.sync.dma_start(
                    ops6[:],
                    opsT[:, 2 * hp : 2 * hp + 2].rearrange("t a p f -> (a p) t f"),
                )
                v8 = v_pool.tile([128, 2, KC, D + 1], bf16, tag="v")
                nc.sync.dma_start(
                    v8[:],
                    vext[2 * hp : 2 * hp + 2].rearrange("h (c p) d -> p h c d", p=128),
                )
                for hh in range(2):
                    h = 2 * hp + hh
                    lo, hi = 64 * hh, 64 * hh + 64
                    po = po_pool.tile([D + 1, S], f32, tag="po")
                    su = su_pool.tile([128, S], f32, tag="su")
                    for p, (ia, ib) in enumerate(PIPES):
                        et4 = et_pool.tile([128, KC, S], bf16, tag="et",
                                           name=f"et4_{p}")
                        for half in range(2):
                            ps = ps_pool.tile([128, 2, S], f32, tag="ps")
                            for sub in range(2):
                                kc = 2 * half + sub
                                nc.tensor.matmul(
                                    ps[:, sub, :], lhsT=ident, rhs=mt[:, kc, :],
                                    start=True, stop=False,
                                )
                                nc.tensor.matmul(
                                    ps[:, sub, :],
                                    lhsT=ops6[lo:hi, ib, ts(kc, 128)],
                                    rhs=ops6[lo:hi, ia, :],
                                    start=False, stop=True,
                                )
                            nc.scalar.activation(
                                et4[:, 2 * half : 2 * half + 2, :], ps[:], Exp
                            )
                        if p == 0:
                            for kc in range(KC):
                                nc.tensor.matmul(
                                    po[:], lhsT=v8[:, hh, kc, :],
                                    rhs=et4[:, kc, :],
                                    start=(kc == 0), stop=(kc == KC - 1),
                                )
                            sums_row = po[D : D + 1, :]
                        else:
                            j = p - 1
                            for kc in range(KC):
                                nc.tensor.matmul(
                                    su[32 * j : 32 * j + 1, :], lhsT=ones_col,
                                    rhs=et4[:, kc, :],
                                    start=(kc == 0), stop=(kc == KC - 1),
                                    tile_position=(0, 32 * j),
                                )
                            sums_row = su[32 * j : 32 * j + 1, :]
                        sums_sb = rec_pool.tile([1, S], f32, tag="sums",
                                                name=f"sums_sb{p}")
                        nc.vector.tensor_copy(sums_sb[:], sums_row)
                        rec_f = rec_pool.tile([1, S], f32, tag="recf",
                                              name=f"rec_f{p}")
                        nc.vector.reciprocal_approx_fast(rec_f[:], sums_sb[:])
                        rec = rec_pool.tile([1, S], bf16, tag="rec",
                                            name=f"rec_{p}")
                        nc.gpsimd.tensor_copy(rec[:], rec_f[:])
                        pbs = pbs_pool.tile([128, S], bf16, tag="pbs",
                                            name=f"pbs_{p}")
                        nc.gpsimd.partition_broadcast(pbs[:], rec[:])
                        pt4 = pt_pool.tile([128, KC, S], bf16, tag="pt",
                                           name=f"pt4_{p}")
                        nc.vector.tensor_mul(
                            pt4[:], et4[:],
                            pbs[:, None, :].broadcast_to([128, KC, S]),
                        )
                        nc.sync.dma_start(pT[p, h][:, 0:2, :], pt4[:, 0:2, :])
                        nc.sync.dma_start(pT[p, h][:, 2:4, :], pt4[:, 2:4, :])
                        if p == 0:
                            oc = oc_pool.tile([D, S], f32, tag="oc")
                            nc.scalar.copy(oc[:], po[0:D, :])
                            on = on_pool.tile([D, S], f32, tag="on")
                            nc.vector.tensor_mul(on[:], oc[:], pbs[0:D, :])
                            nc.sync.dma_start(outT[h], on[:])
    nc.finalize()
    return nc


def _get_nc():
    if "nc" not in _CACHE:
        _CACHE["nc"] = _build_nc()
    return _CACHE["nc"]


def _prep_core_inputs(b, query, key, value, mask, x_original1, x_original2,
                      x_position1, x_position2):
    bf = ml_dtypes.bfloat16
    opdt = bf if BF16_OPS else np.float32
    scale = np.float32(1.0 / np.sqrt(D))

    def t_(x, s=False):
        x = x[b].astype(np.float32)
        if s:
            x = x * scale
        return np.ascontiguousarray(x.transpose(0, 2, 1))  # [H, D, S]

    opsT = np.ascontiguousarray(np.stack([
        t_(query, True), t_(key),
        t_(x_original1, True), t_(x_original2),
        t_(x_position1, True), t_(x_position2),
    ]).astype(opdt))  # [6, H, D, S]
    vext = np.ascontiguousarray(np.concatenate(
        [value[b].astype(np.float32), np.ones((H, S, 1), np.float32)], axis=-1
    ).astype(bf))  # [H, S, D+1]
    mb = (mask[b, 0].astype(np.float32) - 1.0) * np.float32(1e9)  # [q, k]
    maskTb = np.ascontiguousarray(mb.T.astype(bf))
    consts = np.ascontiguousarray(np.concatenate(
        [np.eye(128, dtype=np.float32), np.ones((128, 128), np.float32)], axis=1
    ).astype(bf))
    return dict(opsT=opsT, vext=vext, maskTb=maskTb, consts=consts)


def kernel(query, key, value, mask, x_original1, x_original2, x_position1,
           x_position2, _run_kwargs=None):
    from concourse.bass_utils import run_bass_kernel_spmd

    nc = _get_nc()
    in_maps = [
        _prep_core_inputs(b, query, key, value, mask, x_original1, x_original2,
                          x_position1, x_position2)
        for b in range(B)
    ]
    kw = _run_kwargs or {}
    res = run_bass_kernel_spmd(nc, in_maps, list(range(NCORES)), **kw)
    _CACHE["last_result"] = res

    out = np.empty((B, H, S, D), np.float32)
    ps = [np.empty((B, H, S, S), np.float32) for _ in range(5)]
    for b in range(B):
        r = res.results[b]
        out[b] = np.asarray(r["outT"]).transpose(0, 2, 1)
        pTb = np.asarray(r["pT"]).astype(np.float32)  # [5, H, 128, KC, S]
        # k-major: pTb[j,h,p,c,q] = P[q, c*128+p]
        for j in range(5):
            ps[j][b] = pTb[j].transpose(0, 3, 2, 1).reshape(H, S, S)
    return (out, ps[0], ps[1], ps[2], ps[3], ps[4])


# revision 31
# speedup vs baseline: 1.3639x; 1.3078x over previous
"""Trainium2 Bass kernel for nn_Attention_23433341567267 (sparse_attention).

5 masked-softmax score pipelines over (B=8, H=12, S=512, D=64) plus one
attention-output matmul.  Sharded: core b handles batch b (all 12 heads).

All pipelines are computed k-major (transposed scores):
  sT[k,q] = B[k,:].A[q,:] + maskbiasT[k,q]  (PE: score MM head-pair row-packed
                                             + bf16 identity@maskT inject,
                                             both accumulated in PSUM)
  ET      = exp(sT)                          (ACT -> bf16, 2-chunk ops)
  sums[q]: pipeline 0 via V'=[V|1] ones column inside the PV matmul
           (outT'[65,512] row 64); pipelines 1-4 via ones-column matmuls
           col-packed 4-wide into one PSUM bank (rows 0/32/64/96)
  rec     = 1/sums            (DVE row reciprocal -> bf16)
  pbs     = broadcast(rec)    (GPSIMD partition_broadcast, SBUF bf16)
  PT      = ET * pbs          (DVE/GPSIMD tensor_mul, bf16; DMA out)
  outT    = outT'[0:64] * pbs (ACT copy + DVE mul, f32; DMA out)

Host side does sharding/layout only: d-major transposes, 1/sqrt(D) folded
into the q-side operands, bf16 storage conversion, mask -> additive bias,
and transpose-back + f32 upcast of gathered outputs.
"""

import numpy as np
import ml_dtypes

B, H, S, D = 8, 12, 512, 64
NCORES = 8
KC = S // 128  # 128-chunks per sequence
# (A_idx, B_idx) into the stacked operand tensor
# [0]=qT*scale [1]=kT [2]=xo1T*scale [3]=xo2T [4]=xp1T*scale [5]=xp2T
PIPES = [(0, 1), (2, 3), (2, 5), (4, 3), (4, 5)]
BF16_OPS = True   # score operands in bf16 (f32r otherwise)
POOL_MULS = 0     # of every 10 normalize muls, this many go to GPSIMD

_CACHE = {}


def _build_nc():
    import concourse.mybir as mybir
    import concourse.tile as tile
    from concourse import bacc
    from concourse.bass import ts

    f32 = mybir.dt.float32
    f32r = mybir.dt.float32r
    bf16 = mybir.dt.bfloat16
    opdt = bf16 if BF16_OPS else f32r
    Exp = mybir.ActivationFunctionType.Exp

    nc = bacc.Bacc("TRN2", target_bir_lowering=False, debug=False,
                   num_devices=NCORES)
    opsT = nc.declare_dram_parameter("opsT", [6, H, D, S], opdt, isOutput=False)
    vext = nc.declare_dram_parameter("vext", [H, S, D + 1], bf16, isOutput=False)
    maskTb = nc.declare_dram_parameter("maskTb", [S, S], bf16, isOutput=False)
    consts = nc.declare_dram_parameter("consts", [128, 256], bf16, isOutput=False)
    pT = nc.declare_dram_parameter("pT", [5, H, 128, KC, S], bf16, isOutput=True)
    outT = nc.declare_dram_parameter("outT", [H, D, S], f32, isOutput=True)

    n_mul = 0  # running index to split normalize muls DVE/Pool

    with tile.TileContext(nc) as tc:
        with (
            tc.tile_pool(name="const", bufs=1) as const_pool,
            tc.tile_pool(name="ops", bufs=3) as ops_pool,
            tc.tile_pool(name="v", bufs=3) as v_pool,
            tc.tile_pool(name="et", bufs=15) as et_pool,
            tc.tile_pool(name="pt", bufs=12) as pt_pool,
            tc.tile_pool(name="rec", bufs=8) as rec_pool,
            tc.tile_pool(name="pbs", bufs=8) as pbs_pool,
            tc.tile_pool(name="oc", bufs=2) as oc_pool,
            tc.tile_pool(name="on", bufs=2) as on_pool,
            tc.tile_pool(name="ps", bufs=3, space="PSUM") as ps_pool,
            tc.tile_pool(name="po", bufs=1, space="PSUM") as po_pool,
            tc.tile_pool(name="su", bufs=1, space="PSUM") as su_pool,
        ):
            ct = const_pool.tile([128, 256], bf16)
            nc.sync.dma_start(ct[:], consts[:])
            mt = const_pool.tile([128, KC, S], bf16)
            nc.sync.dma_start(mt[:], maskTb.rearrange("(c p) q -> p c q", p=128))
            ident = ct[:, 0:128]
            ones_col = ct[:, 128:129]

            for hp in range(H // 2):
                ops6 = ops_pool.tile([128, 6, S], opdt, tag="ops")
                nc.sync.dma_start(
                    ops6[:],
                    opsT[:, 2 * hp : 2 * hp + 2].rearrange("t a p f -> (a p) t f"),
                )
                v8 = v_pool.tile([128, 2, KC, D + 1], bf16, tag="v")
                nc.sync.dma_start(
                    v8[:],
                    vext[2 * hp : 2 * hp + 2].rearrange("h (c p) d -> p h c d", p=128),
                )
                for hh in range(2):
                    h = 2 * hp + hh
                    lo, hi = 64 * hh, 64 * hh + 64
                    po = po_pool.tile([D + 1, S], f32, tag="po")
                    su = su_pool.tile([128, S], f32, tag="su")
                    ets = {}

                    def _chain(p, sums_row, ets=ets, po=po, h=h):
                        sums_sb = rec_pool.tile([1, S], f32, tag="sums",
                                                name=f"sums_sb{p}")
                        nc.vector.tensor_copy(sums_sb[:], sums_row)
                        rec_f = rec_pool.tile([1, S], f32, tag="recf",
                                              name=f"rec_f{p}")
                        nc.vector.reciprocal_approx_fast(rec_f[:], sums_sb[:])
                        rec = rec_pool.tile([1, S], bf16, tag="rec",
                                            name=f"rec_{p}")
                        nc.vector.tensor_copy(rec[:], rec_f[:])
                        pbs = pbs_pool.tile([128, S], bf16, tag="pbs",
                                            name=f"pbs_{p}")
                        nc.gpsimd.partition_broadcast(pbs[:], rec[:])
                        pt4 = pt_pool.tile([128, KC, S], bf16, tag="pt",
                                           name=f"pt4_{p}")
                        nc.vector.tensor_mul(
                            pt4[:], ets[p][:],
                            pbs[:, None, :].broadcast_to([128, KC, S]),
                        )
                        nc.sync.dma_start(pT[p, h][:, 0:2, :], pt4[:, 0:2, :])
                        nc.sync.dma_start(pT[p, h][:, 2:4, :], pt4[:, 2:4, :])
                        if p == 0:
                            oc = oc_pool.tile([D, S], f32, tag="oc")
                            nc.scalar.copy(oc[:], po[0:D, :])
                            on = on_pool.tile([D, S], f32, tag="on")
                            nc.vector.tensor_mul(on[:], oc[:], pbs[0:D, :])
                            nc.sync.dma_start(outT[h], on[:])

                    for p, (ia, ib) in enumerate(PIPES):
                        et4 = et_pool.tile([128, KC, S], bf16, tag="et",
                                           name=f"et4_{p}")
                        for half in range(2):
                            ps = ps_pool.tile([128, 2, S], f32, tag="ps")
                            for sub in range(2):
                                kc = 2 * half + sub
                                nc.tensor.matmul(
                                    ps[:, sub, :], lhsT=ident, rhs=mt[:, kc, :],
                                    start=True, stop=False,
                                )
                                nc.tensor.matmul(
                                    ps[:, sub, :],
                                    lhsT=ops6[lo:hi, ib, ts(kc, 128)],
                                    rhs=ops6[lo:hi, ia, :],
                                    start=False, stop=True,
                                )
                            nc.scalar.activation(
                                et4[:, 2 * half : 2 * half + 2, :], ps[:], Exp
                            )
                        ets[p] = et4
                        if p == 0:
                            for kc in range(KC):
                                nc.tensor.matmul(
                                    po[:], lhsT=v8[:, hh, kc, :],
                                    rhs=et4[:, kc, :],
                                    start=(kc == 0), stop=(kc == KC - 1),
                                )
                            _chain(0, po[D : D + 1, :])
                        elif p in (2, 4):
                            # paired sums: adjacent MMs for (p-1, p) land in
                            # disjoint PSUM col groups -> 2-wide concurrent
                            for kc in range(KC):
                                for pj in (p - 1, p):
                                    j = pj - 1
                                    nc.tensor.matmul(
                                        su[32 * j : 32 * j + 1, :],
                                        lhsT=ones_col,
                                        rhs=ets[pj][:, kc, :],
                                        start=(kc == 0), stop=(kc == KC - 1),
                                        tile_position=(0, 32 * j),
                                    )
                            _chain(p - 1, su[32 * (p - 2) : 32 * (p - 2) + 1, :])
                            _chain(p, su[32 * (p - 1) : 32 * (p - 1) + 1, :])
    nc.finalize()
    return nc


def _get_nc():
    if "nc" not in _CACHE:
        _CACHE["nc"] = _build_nc()
    return _CACHE["nc"]


def _prep_core_inputs(b, query, key, value, mask, x_original1, x_original2,
                      x_position1, x_position2):
    bf = ml_dtypes.bfloat16
    opdt = bf if BF16_OPS else np.float32
    scale = np.float32(1.0 / np.sqrt(D))

    def t_(x, s=False):
        x = x[b].astype(np.float32)
        if s:
            x = x * scale
        return np.ascontiguousarray(x.transpose(0, 2, 1))  # [H, D, S]

    opsT = np.ascontiguousarray(np.stack([
        t_(query, True), t_(key),
        t_(x_original1, True), t_(x_original2),
        t_(x_position1, True), t_(x_position2),
    ]).astype(opdt))  # [6, H, D, S]
    vext = np.ascontiguousarray(np.concatenate(
        [value[b].astype(np.float32), np.ones((H, S, 1), np.float32)], axis=-1
    ).astype(bf))  # [H, S, D+1]
    mb = (mask[b, 0].astype(np.float32) - 1.0) * np.float32(1e9)  # [q, k]
    maskTb = np.ascontiguousarray(mb.T.astype(bf))
    consts = np.ascontiguousarray(np.concatenate(
        [np.eye(128, dtype=np.float32), np.ones((128, 128), np.float32)], axis=1
    ).astype(bf))
    return dict(opsT=opsT, vext=vext, maskTb=maskTb, consts=consts)


def kernel(query, key, value, mask, x_original1, x_original2, x_position1,
           x_position2, _run_kwargs=None):
    from concourse.bass_utils import run_bass_kernel_spmd

    nc = _get_nc()
    in_maps = [
        _prep_core_inputs(b, query, key, value, mask, x_original1, x_original2,
                          x_position1, x_position2)
        for b in range(B)
    ]
    kw = _run_kwargs or {}
    res = run_bass_kernel_spmd(nc, in_maps, list(range(NCORES)), **kw)
    _CACHE["last_result"] = res

    out = np.empty((B, H, S, D), np.float32)
    ps = [np.empty((B, H, S, S), np.float32) for _ in range(5)]
    for b in range(B):
        r = res.results[b]
        out[b] = np.asarray(r["outT"]).transpose(0, 2, 1)
        pTb = np.asarray(r["pT"]).astype(np.float32)  # [5, H, 128, KC, S]
        # k-major: pTb[j,h,p,c,q] = P[q, c*128+p]
        for j in range(5):
            ps[j][b] = pTb[j].transpose(0, 3, 2, 1).reshape(H, S, S)
    return (out, ps[0], ps[1], ps[2], ps[3], ps[4])


# revision 33
# speedup vs baseline: 1.5385x; 1.1281x over previous
"""Trainium2 Bass kernel for nn_Attention_23433341567267 (sparse_attention).

5 masked-softmax score pipelines over (B=8, H=12, S=512, D=64) plus one
attention-output matmul.  Sharded: core b handles batch b (all 12 heads).

All pipelines are computed k-major (transposed scores):
  sT[k,q] = B[k,:].A[q,:] + maskbiasT[k,q]  (PE: score MM head-pair row-packed
                                             + bf16 identity@maskT inject,
                                             both accumulated in PSUM)
  ET      = exp(sT)                          (ACT -> bf16, 2-chunk ops)
  sums[q]: pipeline 0 via V'=[V|1] ones column inside the PV matmul
           (outT'[65,512] row 64); pipelines 1-4 via ones-column matmuls
           col-packed 4-wide into one PSUM bank (rows 0/32/64/96)
           (pairs (p1,p2)/(p3,p4) emitted adjacently -> 2-wide col-group
           concurrency on the PE)
  rec     = 1/sums            (DVE copy + reciprocal_approx_fast + bf16 cast)
  pbs     = broadcast(rec)    (GPSIMD partition_broadcast, SBUF bf16)
  PT      = ET * pbs          (DVE tensor_mul bf16 2x; split DMA out)
  outT    = outT'[0:64] * pbs (ACT copy + DVE mul, f32; DMA out)
Each pipeline's sums/normalize/store chain is emitted right after its exps
(software pipelining at pipeline granularity keeps the PE warm).

Host side does sharding/layout only: d-major transposes, 1/sqrt(D) folded
into the q-side operands, bf16 storage conversion, mask -> additive bias,
and transpose-back + f32 upcast of gathered outputs.
"""

import numpy as np
import ml_dtypes

B, H, S, D = 8, 12, 512, 64
NCORES = 8
KC = S // 128  # 128-chunks per sequence
# (A_idx, B_idx) into the stacked operand tensor
# [0]=qT*scale [1]=kT [2]=xo1T*scale [3]=xo2T [4]=xp1T*scale [5]=xp2T
PIPES = [(0, 1), (2, 3), (2, 5), (4, 3), (4, 5)]
BF16_OPS = True   # score operands in bf16 (f32r otherwise)
POOL_MULS = 0     # of every 10 normalize muls, this many go to GPSIMD

_CACHE = {}


def _build_nc():
    import concourse.mybir as mybir
    import concourse.tile as tile
    from concourse import bacc
    from concourse.bass import ts

    f32 = mybir.dt.float32
    f32r = mybir.dt.float32r
    bf16 = mybir.dt.bfloat16
    opdt = bf16 if BF16_OPS else f32r
    Exp = mybir.ActivationFunctionType.Exp

    nc = bacc.Bacc("TRN2", target_bir_lowering=False, debug=False,
                   num_devices=NCORES)
    opsT = nc.declare_dram_parameter("opsT", [6, H, D, S], opdt, isOutput=False)
    vext = nc.declare_dram_parameter("vext", [H, S, D + 1], bf16, isOutput=False)
    maskTb = nc.declare_dram_parameter("maskTb", [S, S], bf16, isOutput=False)
    consts = nc.declare_dram_parameter("consts", [128, 256], bf16, isOutput=False)
    pT = nc.declare_dram_parameter("pT", [5, H, 128, KC, S], bf16, isOutput=True)
    outT = nc.declare_dram_parameter("outT", [H, D, S], f32, isOutput=True)

    n_mul = 0  # running index to split normalize muls DVE/Pool

    with tile.TileContext(nc) as tc:
        with (
            tc.tile_pool(name="const", bufs=1) as const_pool,
            tc.tile_pool(name="ops", bufs=2) as ops_pool,
            tc.tile_pool(name="v", bufs=3) as v_pool,
            tc.tile_pool(name="et", bufs=14) as et_pool,
            tc.tile_pool(name="pt", bufs=12) as pt_pool,
            tc.tile_pool(name="rec", bufs=6) as rec_pool,
            tc.tile_pool(name="pbs", bufs=8) as pbs_pool,
            tc.tile_pool(name="oc", bufs=2) as oc_pool,
            tc.tile_pool(name="on", bufs=2) as on_pool,
            tc.tile_pool(name="ps", bufs=3, space="PSUM") as ps_pool,
            tc.tile_pool(name="po", bufs=1, space="PSUM") as po_pool,
            tc.tile_pool(name="su", bufs=1, space="PSUM") as su_pool,
        ):
            ct = const_pool.tile([128, 256], bf16)
            nc.sync.dma_start(ct[:], consts[:])
            mt = const_pool.tile([128, KC, S], bf16)
            nc.sync.dma_start(mt[:], maskTb.rearrange("(c p) q -> p c q", p=128))
            ident = ct[:, 0:128]
            ones_col = ct[:, 128:129]

            for hp in range(H // 2):
                ops6 = ops_pool.tile([128, 6, S], opdt, tag="ops")
                nc.sync.dma_start(
                    ops6[:],
                    opsT[:, 2 * hp : 2 * hp + 2].rearrange("t a p f -> (a p) t f"),
                )
                ops6b = ops_pool.tile([128, 6, S], opdt, tag="opsb")
                nc.sync.dma_start(
                    ops6b[0:64],
                    opsT[:, 2 * hp + 1 : 2 * hp + 2].rearrange("t a p f -> (a p) t f"),
                )
                nc.sync.dma_start(
                    ops6b[64:128],
                    opsT[:, 2 * hp : 2 * hp + 1].rearrange("t a p f -> (a p) t f"),
                )
                v8 = v_pool.tile([128, 2, KC, D + 1], bf16, tag="v")
                nc.sync.dma_start(
                    v8[:],
                    vext[2 * hp : 2 * hp + 2].rearrange("h (c p) d -> p h c d", p=128),
                )
                for hh in range(2):
                    h = 2 * hp + hh
                    lo, hi = 64 * hh, 64 * hh + 64
                    po = po_pool.tile([D + 1, S], f32, tag="po")
                    su = su_pool.tile([128, S], f32, tag="su")
                    ets = {}

                    def _chain(p, sums_row, ets=ets, po=po, h=h):
                        sums_sb = rec_pool.tile([1, S], f32, tag="sums",
                                                name=f"sums_sb{p}")
                        nc.vector.tensor_copy(sums_sb[:], sums_row)
                        rec_f = rec_pool.tile([1, S], f32, tag="recf",
                                              name=f"rec_f{p}")
                        nc.vector.reciprocal_approx_fast(rec_f[:], sums_sb[:])
                        rec = rec_pool.tile([1, S], bf16, tag="rec",
                                            name=f"rec_{p}")
                        nc.vector.tensor_copy(rec[:], rec_f[:])
                        pbs = pbs_pool.tile([128, S], bf16, tag="pbs",
                                            name=f"pbs_{p}")
                        nc.gpsimd.partition_broadcast(pbs[:], rec[:])
                        pt4 = pt_pool.tile([128, KC, S], bf16, tag="pt",
                                           name=f"pt4_{p}")
                        nc.vector.tensor_mul(
                            pt4[:], ets[p][:],
                            pbs[:, None, :].broadcast_to([128, KC, S]),
                        )
                        nc.sync.dma_start(pT[p, h][:, 0:2, :], pt4[:, 0:2, :])
                        nc.sync.dma_start(pT[p, h][:, 2:4, :], pt4[:, 2:4, :])
                        if p == 0:
                            oc = oc_pool.tile([D, S], f32, tag="oc")
                            nc.scalar.copy(oc[:], po[0:D, :])
                            on = on_pool.tile([D, S], f32, tag="on")
                            nc.vector.tensor_mul(on[:], oc[:], pbs[0:D, :])
                            nc.sync.dma_start(outT[h], on[:])

                    for p, (ia, ib) in enumerate(PIPES):
                        et4 = et_pool.tile([128, KC, S], bf16, tag="et",
                                           name=f"et4_{p}")
                        for half in range(2):
                            ps = ps_pool.tile([128, 2, S], f32, tag="ps")
                            for sub in range(2):
                                kc = 2 * half + sub
                                nc.tensor.matmul(
                                    ps[:, sub, :], lhsT=ident, rhs=mt[:, kc, :],
                                    start=True, stop=False,
                                )
                            for sub in range(2):
                                kc = 2 * half + sub
                                if sub == 0:
                                    o, l2 = ops6, 64 * hh
                                else:
                                    o, l2 = ops6b, 64 * (1 - hh)
                                nc.tensor.matmul(
                                    ps[:, sub, :],
                                    lhsT=o[l2 : l2 + 64, ib, ts(kc, 128)],
                                    rhs=o[l2 : l2 + 64, ia, :],
                                    start=False, stop=True,
                                )
                            nc.scalar.activation(
                                et4[:, 2 * half : 2 * half + 2, :], ps[:], Exp
                            )
                        ets[p] = et4
                        if p == 0:
                            for kc in range(KC):
                                nc.tensor.matmul(
                                    po[:], lhsT=v8[:, hh, kc, :],
                                    rhs=et4[:, kc, :],
                                    start=(kc == 0), stop=(kc == KC - 1),
                                )
                            _chain(0, po[D : D + 1, :])
                        elif p in (2, 4):
                            # paired sums: adjacent MMs for (p-1, p) land in
                            # disjoint PSUM col groups -> 2-wide concurrent
                            for kc in range(KC):
                                for pj in (p - 1, p):
                                    j = pj - 1
                                    nc.tensor.matmul(
                                        su[32 * j : 32 * j + 1, :],
                                        lhsT=ones_col,
                                        rhs=ets[pj][:, kc, :],
                                        start=(kc == 0), stop=(kc == KC - 1),
                                        tile_position=(0, 32 * j),
                                    )
                            _chain(p - 1, su[32 * (p - 2) : 32 * (p - 2) + 1, :])
                            _chain(p, su[32 * (p - 1) : 32 * (p - 1) + 1, :])
    nc.finalize()
    return nc


def _get_nc():
    if "nc" not in _CACHE:
        _CACHE["nc"] = _build_nc()
    return _CACHE["nc"]


def _prep_core_inputs(b, query, key, value, mask, x_original1, x_original2,
                      x_position1, x_position2):
    bf = ml_dtypes.bfloat16
    opdt = bf if BF16_OPS else np.float32
    scale = np.float32(1.0 / np.sqrt(D))

    def t_(x, s=False):
        x = x[b].astype(np.float32)
        if s:
            x = x * scale
        return np.ascontiguousarray(x.transpose(0, 2, 1))  # [H, D, S]

    opsT = np.ascontiguousarray(np.stack([
        t_(query, True), t_(key),
        t_(x_original1, True), t_(x_original2),
        t_(x_position1, True), t_(x_position2),
    ]).astype(opdt))  # [6, H, D, S]
    vext = np.ascontiguousarray(np.concatenate(
        [value[b].astype(np.float32), np.ones((H, S, 1), np.float32)], axis=-1
    ).astype(bf))  # [H, S, D+1]
    mb = (mask[b, 0].astype(np.float32) - 1.0) * np.float32(1e9)  # [q, k]
    maskTb = np.ascontiguousarray(mb.T.astype(bf))
    consts = np.ascontiguousarray(np.concatenate(
        [np.eye(128, dtype=np.float32), np.ones((128, 128), np.float32)], axis=1
    ).astype(bf))
    return dict(opsT=opsT, vext=vext, maskTb=maskTb, consts=consts)


def kernel(query, key, value, mask, x_original1, x_original2, x_position1,
           x_position2, _run_kwargs=None):
    from concourse.bass_utils import run_bass_kernel_spmd

    nc = _get_nc()
    in_maps = [
        _prep_core_inputs(b, query, key, value, mask, x_original1, x_original2,
                          x_position1, x_position2)
        for b in range(B)
    ]
    kw = _run_kwargs or {}
    res = run_bass_kernel_spmd(nc, in_maps, list(range(NCORES)), **kw)
    _CACHE["last_result"] = res

    out = np.empty((B, H, S, D), np.float32)
    ps = [np.empty((B, H, S, S), np.float32) for _ in range(5)]
    for b in range(B):
        r = res.results[b]
        out[b] = np.asarray(r["outT"]).transpose(0, 2, 1)
        pTb = np.asarray(r["pT"]).astype(np.float32)  # [5, H, 128, KC, S]
        # k-major: pTb[j,h,p,c,q] = P[q, c*128+p]
        for j in range(5):
            ps[j][b] = pTb[j].transpose(0, 3, 2, 1).reshape(H, S, S)
    return (out, ps[0], ps[1], ps[2], ps[3], ps[4])
